# revision 18
# baseline (speedup 1.0000x reference)
"""Trainium2 Bass kernel for nn_DiagSSMBlock (T=4096, H=1024, fp32).

Math: s = b_mat.T @ x_seq.T  (H,T);  h[:, t] = a * h[:, t-1] + s[:, t]
      output = h.T  (T, H)

a_diag is glorot-scaled (|a| <= sqrt(2/1024) ~ 0.044): the power kernel decays
below fp32 epsilon within 8 taps, so an 8-step halo makes the T-sharded
recurrence exact to fp32 precision.  The 2e-2 rel-err budget also admits bf16
operands end to end (measured ~4e-3), which halves every DMA byte and doubles
matmul/weight-load throughput paths.

Sharding (8 cores): 4-way along T x 2-way along H_out.
Per core:
  GEMM   (1032 t) x (512 h_out) x (1024 contract) in bf16 via PE matmuls,
         k-outer over h-tiles {m0,m1,m2} while input chunks stream, then m3
         (PSUM: 8 banks = 3x2 segs + halo slivers + warm/B-phase reuse).
  copies PSUM fp32 -> SBUF bf16 on the scalar engine.
  scan   DVE tensor_tensor_scan per h-tile, lo half chained into hi half via
         a tensor `initial` (halo columns warm the carry from zero).
  out    [h, t] layout DMA'd straight from the scan output -- no transposes;
         the host transposes each core's (512, 1024) block while unsharding.
"""

import sys

import numpy as np

if "/opt/trn_rl_repo" not in sys.path:
    sys.path.insert(0, "/opt/trn_rl_repo")

import ml_dtypes

BF16 = ml_dtypes.bfloat16

T, H = 4096, 1024
NC_T, NC_H = 4, 2  # core grid: 4 T-shards x 2 H-shards
TL = T // NC_T  # 1024 output rows per core
HL = H // NC_H  # 512 output cols per core
HALO = 8  # recurrence warm-up steps
TLH = TL + HALO  # 1032
P = 128
KC = H // P  # 8 contraction chunks
MT = HL // P  # 4 h_out tiles per core
N_CORES = NC_T * NC_H

_CACHE = {}


def _build_program():
    from contextlib import ExitStack

    import concourse.bass as bass
    import concourse.tile as tile
    from concourse import bacc, mybir

    f32 = mybir.dt.float32
    bf16 = mybir.dt.bfloat16
    Copy = mybir.ActivationFunctionType.Copy
    ADD = mybir.AluOpType.add
    MULT = mybir.AluOpType.mult

    nc = bacc.Bacc("TRN2", target_bir_lowering=False, debug=False, num_devices=N_CORES)

    # partition-major: row p holds that partition's KC chunks back to back,
    # so multi-chunk DMA groups move one large descriptor per partition.
    xt_d = nc.dram_tensor("xt", [P, KC * TLH], bf16, kind="ExternalInput").ap()
    b_d = nc.dram_tensor("bm", [P, KC * HL], bf16, kind="ExternalInput").ap()
    a_d = nc.dram_tensor("apd", [P, MT], f32, kind="ExternalInput").ap()
    out_d = nc.dram_tensor("out", [HL, TL], bf16, kind="ExternalOutput").ap()

    from concourse.tile import add_dep_helper

    with tile.TileContext(nc) as tc, ExitStack() as ctx:
        const = ctx.enter_context(tc.tile_pool(name="const", bufs=1))
        s_pool = ctx.enter_context(tc.tile_pool(name="s", bufs=1))
        g_pool = ctx.enter_context(tc.tile_pool(name="g", bufs=1))
        psum = ctx.enter_context(tc.tile_pool(name="psfix", bufs=1, space="PSUM"))

        xt_sb = const.tile([P, KC, TLH], bf16)
        b_sb = const.tile([P, KC, HL], bf16)
        a_raw = const.tile([P, MT], f32)
        w_sb = const.tile([P, P], bf16)
        a_rep = [const.tile([P, TLH], bf16, name=f"arep{m}") for m in range(MT)]
        s_sb = [s_pool.tile([P, TLH], bf16, name=f"s{m}") for m in range(MT)]
        g_lo0 = g_pool.tile([P, 520], bf16, name="glo0")
        g_hi0 = g_pool.tile([P, 512], bf16, name="ghi0")
        g_all = [g_pool.tile([P, TLH], bf16, name=f"g{m}") for m in (1, 2)]
        g_lo3 = g_pool.tile([P, 520], bf16, name="glo3")
        g_hi3a = g_pool.tile([P, 256], bf16, name="ghi3a")
        g_hi3b = g_pool.tile([P, 256], bf16, name="ghi3b")

        # warm-matmul operand: on-chip memset, no DMA dependency, so the PE
        # warm-up chain starts the moment the engines come up.
        nc.vector.memset(w_sb[:, :], 0.0)

        # --- input DMAs.  x chunks on sync, b chunks on scalar: two HWDGE
        # FIFO streams drain in issue order, so chunk k lands ~k*1.45us in and
        # the k-outer GEMM consumes right behind the stream.  apd rides SWDGE
        # on the otherwise-idle GpSimd sequencer.
        nc.gpsimd.dma_start(out=a_raw[:, :], in_=a_d[:, :])
        for lo, hi in ((0, 1), (1, 2), (2, 4), (4, 8)):
            nc.sync.dma_start(
                out=xt_sb[:, lo:hi, :],
                in_=xt_d[:, lo * TLH:hi * TLH].rearrange("p (c f) -> p c f", f=TLH),
            )
            nc.scalar.dma_start(
                out=b_sb[:, lo:hi, :],
                in_=b_d[:, lo * HL:hi * HL].rearrange("p (c f) -> p c f", f=HL),
            )

        # a broadcast in bf16, materialized (packed last dim) while DVE is
        # otherwise idle.
        for m in range(MT):
            nc.vector.tensor_copy(
                a_rep[m][:, :], a_raw[:, m:m + 1].broadcast_to([P, TLH])
            )

        ps = [psum.tile([P, 512], f32, tag=f"ps{i}", name=f"ps{i}") for i in range(8)]
        # bank plan: phase A (m0..m2): segs -> ps[2m], ps[2m+1]; halos -> ps6
        # cols [8m:8m+8].  warm matmuls + phase B (m3): seg0 -> ps7,
        # seg1 -> ps0 (freed by first copy), halo -> ps6 cols [24:32].

        # --- PE warmup while the first chunks stream: flips the HAM
        # clock-gate to 8/8 before the real GEMM.
        def warm_mm():
            return nc.tensor.matmul(
                ps[7][0:P, 0:P], lhsT=w_sb[:, :], rhs=w_sb[:, :],
                start=True, stop=True,
            )

        # ~25 warm matmuls bridge from engine-up (~7.5us) to chunk0 arrival
        # (~11.5us) so k0 runs at the full 2.4 GHz clock.
        warm_last = None
        for _ in range(25):
            warm_last = warm_mm()

        def mm(out_ap, k, m, rhs_cols, start, stop):
            r = nc.tensor.matmul(
                out_ap,
                lhsT=b_sb[:, k, m * P:(m + 1) * P],
                rhs=xt_sb[:, k, rhs_cols[0]:rhs_cols[1]],
                start=start,
                stop=stop,
            )
            add_dep_helper(r.ins, warm_last.ins, sync=False)
            return r

        # --- phase A: m0..m2 k-outer (PE consumes ~1.3us per chunk, just
        # above the DMA delivery rate, so the PE stays busy and warm).
        for k in range(KC):
            st, sp = (k == 0), (k == KC - 1)
            for m in range(3):
                mm(ps[2 * m][:, :], k, m, (HALO, HALO + 512), st, sp)
                mm(ps[2 * m + 1][:, :], k, m, (HALO + 512, TLH), st, sp)
                mm(ps[6][:, 8 * m:8 * m + 8], k, m, (0, HALO), st, sp)
            if k < KC - 1:
                warm_mm()  # keep the HAM gate open across chunk-arrival gaps

        # --- phase A copies (scalar engine), ordered so the banks phase B
        # needs free up first: ps0 (B seg1), then the ps6 halo slivers
        # (B halo), then the rest; m0's three land first so its scans start
        # immediately.
        def cp(dst, src):
            nc.scalar.activation(dst, src, Copy)

        cp(s_sb[0][:, HALO:HALO + 512], ps[0][:, :])
        cp(s_sb[0][:, 0:HALO], ps[6][:, 0:8])
        cp(s_sb[0][:, HALO + 512:TLH], ps[1][:, :])
        cp(s_sb[1][:, 0:HALO], ps[6][:, 8:16])
        cp(s_sb[2][:, 0:HALO], ps[6][:, 16:24])
        cp(s_sb[1][:, HALO:HALO + 512], ps[2][:, :])
        cp(s_sb[1][:, HALO + 512:TLH], ps[3][:, :])
        cp(s_sb[2][:, HALO:HALO + 512], ps[4][:, :])
        cp(s_sb[2][:, HALO + 512:TLH], ps[5][:, :])

        # --- phase B: m3, seg-major (all chunks are resident by now).
        for k in range(KC):
            mm(ps[7][:, :], k, 3, (HALO, HALO + 512), k == 0, k == KC - 1)
        for k in range(KC):
            mm(ps[0][:, :], k, 3, (HALO + 512, TLH), k == 0, k == KC - 1)
        for k in range(KC):
            mm(ps[6][:, 24:32], k, 3, (0, HALO), k == 0, k == KC - 1)

        cp(s_sb[3][:, HALO:HALO + 512], ps[7][:, :])
        cp(s_sb[3][:, 0:HALO], ps[6][:, 24:32])
        cp(s_sb[3][:, HALO + 512:TLH], ps[0][:, :])

        # --- scans (DVE) + out DMAs (sync).  The first 8 columns warm the
        # carry from 0 and are discarded.  m0 is split lo/hi so its first
        # scan only waits for two copies; m1/m2 are single 1032-col scans;
        # m3 ends in two 256-col scans so the final out transfer is small
        # and the previous out overlaps the last scan.
        scan = nc.vector.tensor_tensor_scan
        scan(g_lo0[:, :], a_rep[0][:, 0:520], s_sb[0][:, 0:520], 0.0, MULT, ADD)
        nc.sync.dma_start(out=out_d[0:P, 0:512], in_=g_lo0[:, HALO:520])
        scan(g_hi0[:, :], a_rep[0][:, 0:512], s_sb[0][:, 520:TLH],
             g_lo0[:, 519:520], MULT, ADD)
        nc.sync.dma_start(out=out_d[0:P, 512:TL], in_=g_hi0[:, :])
        for i, m in enumerate((1, 2)):
            scan(g_all[i][:, :], a_rep[m][:, :], s_sb[m][:, :], 0.0, MULT, ADD)
            nc.sync.dma_start(
                out=out_d[m * P:(m + 1) * P, :], in_=g_all[i][:, HALO:TLH]
            )
        scan(g_lo3[:, :], a_rep[3][:, 0:520], s_sb[3][:, 0:520], 0.0, MULT, ADD)
        nc.sync.dma_start(out=out_d[3 * P:4 * P, 0:512], in_=g_lo3[:, HALO:520])
        scan(g_hi3a[:, :], a_rep[3][:, 0:256], s_sb[3][:, 520:776],
             g_lo3[:, 519:520], MULT, ADD)
        nc.sync.dma_start(out=out_d[3 * P:4 * P, 512:768], in_=g_hi3a[:, :])
        scan(g_hi3b[:, :], a_rep[3][:, 0:256], s_sb[3][:, 776:TLH],
             g_hi3a[:, 255:256], MULT, ADD)
        nc.sync.dma_start(out=out_d[3 * P:4 * P, 768:TL], in_=g_hi3b[:, :])

    nc.compile()
    return nc


def _get_nc():
    if "nc" not in _CACHE:
        _CACHE["nc"] = _build_program()
    return _CACHE["nc"]


def _make_in_maps(x_seq, a_diag, b_mat):
    x_seq = np.ascontiguousarray(x_seq, dtype=np.float32)
    a_diag = np.asarray(a_diag, dtype=np.float32)
    b_mat = np.ascontiguousarray(b_mat, dtype=np.float32)

    # (H, HALO+T) in bf16: zero left-pad so every core reads [t0-8, t0+TL)
    xtp = np.concatenate(
        [np.zeros((H, HALO), np.float32), x_seq.T], axis=1
    ).astype(BF16)
    b16 = b_mat.astype(BF16)

    in_maps = []
    for c in range(N_CORES):
        ct, ch = divmod(c, NC_H)
        t0 = ct * TL
        h0 = ch * HL
        a_loc = a_diag[h0:h0 + HL].reshape(MT, P).T  # (128, MT)
        # partition-major: row p = chunks k=0..7 of partition p back to back
        xt_pm = (
            xtp[:, t0:t0 + TLH].reshape(KC, P, TLH)
            .transpose(1, 0, 2).reshape(P, KC * TLH)
        )
        b_pm = (
            b16[:, h0:h0 + HL].reshape(KC, P, HL)
            .transpose(1, 0, 2).reshape(P, KC * HL)
        )
        in_maps.append({
            "xt": np.ascontiguousarray(xt_pm),
            "bm": np.ascontiguousarray(b_pm),
            "apd": np.ascontiguousarray(a_loc),
        })
    return in_maps


def _run(x_seq, a_diag, b_mat, trace=False):
    from concourse.bass_utils import run_bass_kernel_spmd

    nc = _get_nc()
    in_maps = _make_in_maps(x_seq, a_diag, b_mat)
    res = run_bass_kernel_spmd(nc, in_maps, list(range(N_CORES)), trace=trace)

    out = np.empty((T, H), np.float32)
    for c in range(N_CORES):
        ct, ch = divmod(c, NC_H)
        # per-core result is (HL, TL) bf16 in [h, t] layout
        blk = np.asarray(res.results[c]["out"], dtype=np.float32)
        out[ct * TL:(ct + 1) * TL, ch * HL:(ch + 1) * HL] = blk.T
    return out, res


def kernel(x_seq, a_diag, b_mat):
    out, _ = _run(x_seq, a_diag, b_mat, trace=False)
    return out


# revision 24
# speedup vs baseline: 1.1186x; 1.1186x over previous
"""Trainium2 Bass kernel for nn_DiagSSMBlock (T=4096, H=1024, fp32).

Math: s = b_mat.T @ x_seq.T  (H,T);  h[:, t] = a * h[:, t-1] + s[:, t]
      output = h.T  (T, H)

a_diag is glorot-scaled (|a| <= sqrt(2/1024) ~ 0.044): the power kernel decays
below fp32 epsilon within 8 taps, so an 8-step halo makes the T-sharded
recurrence exact to fp32 precision.  The 2e-2 rel-err budget also admits bf16
operands end to end (measured ~4e-3), which halves every DMA byte and doubles
matmul/weight-load throughput paths.

Sharding (8 cores): 4-way along T x 2-way along H_out.
Per core:
  GEMM   (1032 t) x (512 h_out) x (1024 contract) in bf16 via PE matmuls,
         k-outer over h-tiles {m0,m1,m2} while input chunks stream, then m3
         (PSUM: 8 banks = 3x2 segs + halo slivers + warm/B-phase reuse).
  copies PSUM fp32 -> SBUF bf16 on the scalar engine.
  scan   DVE tensor_tensor_scan per h-tile, lo half chained into hi half via
         a tensor `initial` (halo columns warm the carry from zero).
  out    [h, t] layout DMA'd straight from the scan output -- no transposes;
         the host transposes each core's (512, 1024) block while unsharding.
"""

import sys

import numpy as np

if "/opt/trn_rl_repo" not in sys.path:
    sys.path.insert(0, "/opt/trn_rl_repo")

import ml_dtypes

BF16 = ml_dtypes.bfloat16

T, H = 4096, 1024
NC_T, NC_H = 4, 2  # core grid: 4 T-shards x 2 H-shards
TL = T // NC_T  # 1024 output rows per core
HL = H // NC_H  # 512 output cols per core
HALO = 8  # recurrence warm-up steps
TLH = TL + HALO  # 1032
P = 128
KC = H // P  # 8 contraction chunks
MT = HL // P  # 4 h_out tiles per core
N_CORES = NC_T * NC_H

_CACHE = {}


def _build_program():
    from contextlib import ExitStack

    import concourse.bass as bass
    import concourse.tile as tile
    from concourse import bacc, mybir

    f32 = mybir.dt.float32
    bf16 = mybir.dt.bfloat16
    Copy = mybir.ActivationFunctionType.Copy
    ADD = mybir.AluOpType.add
    MULT = mybir.AluOpType.mult

    nc = bacc.Bacc("TRN2", target_bir_lowering=False, debug=False, num_devices=N_CORES)

    xt_d = nc.dram_tensor("xt", [H, TLH], bf16, kind="ExternalInput").ap()
    b_d = nc.dram_tensor("bm", [H, HL], bf16, kind="ExternalInput").ap()
    a_d = nc.dram_tensor("apd", [P, MT], f32, kind="ExternalInput").ap()
    out_d = nc.dram_tensor("out", [HL, TL], bf16, kind="ExternalOutput").ap()

    from concourse.tile import add_dep_helper

    with tile.TileContext(nc) as tc, ExitStack() as ctx:
        const = ctx.enter_context(tc.tile_pool(name="const", bufs=1))
        s_pool = ctx.enter_context(tc.tile_pool(name="s", bufs=1))
        g_pool = ctx.enter_context(tc.tile_pool(name="g", bufs=1))
        psum = ctx.enter_context(tc.tile_pool(name="psfix", bufs=1, space="PSUM"))

        xt_sb = const.tile([P, KC, TLH], bf16)
        b_sb = const.tile([P, KC, HL], bf16)
        a_raw = const.tile([P, MT], f32)
        w_sb = const.tile([P, P], bf16)
        a_rep = [const.tile([P, TLH], bf16, name=f"arep{m}") for m in range(MT)]
        s_sb = [s_pool.tile([P, TLH], bf16, name=f"s{m}") for m in range(MT)]
        g_all = [g_pool.tile([P, TLH], bf16, name=f"g{m}") for m in range(3)]
        g_lo3 = g_pool.tile([P, 520], bf16, name="glo3")
        g_hi3 = g_pool.tile([P, 512], bf16, name="ghi3")

        # warm-matmul operand: on-chip memset, no DMA dependency, so the PE
        # warm-up chain starts the moment the engines come up.
        nc.vector.memset(w_sb[:, :], 0.0)

        # --- input DMAs on THREE rings (x-even on sync, x-odd on scalar,
        # b on GpSimd's SWDGE): the SDMA engines were ~68% busy with two
        # FIFO streams (HBM-latency stalls); a third stream deepens the
        # per-engine queues and closes the gap.
        nc.gpsimd.dma_start(out=a_raw[:, :], in_=a_d[:, :])
        for k in range(KC):
            xeng = nc.sync if k % 2 == 0 else nc.scalar
            xeng.dma_start(out=xt_sb[:, k, :], in_=xt_d[k * P:(k + 1) * P, :])
            nc.gpsimd.dma_start(out=b_sb[:, k, :], in_=b_d[k * P:(k + 1) * P, :])

        # a broadcast in bf16, materialized (packed last dim) while DVE is
        # otherwise idle.
        for m in range(MT):
            nc.vector.tensor_copy(
                a_rep[m][:, :], a_raw[:, m:m + 1].broadcast_to([P, TLH])
            )

        ps = [psum.tile([P, 512], f32, tag=f"ps{i}", name=f"ps{i}") for i in range(8)]
        # bank plan: phase A (m0..m2): segs -> ps[2m], ps[2m+1]; halos -> ps6
        # cols [8m:8m+8].  warm matmuls + phase B (m3): seg0 -> ps7,
        # seg1 -> ps0 (freed by first copy), halo -> ps6 cols [24:32].

        # --- PE warmup while the first chunks stream: flips the HAM
        # clock-gate to 8/8 before the real GEMM.
        def warm_mm():
            return nc.tensor.matmul(
                ps[7][0:P, 0:P], lhsT=w_sb[:, :], rhs=w_sb[:, :],
                start=True, stop=True,
            )

        # ~25 warm matmuls bridge from engine-up (~7.5us) to chunk0 arrival
        # (~11.5us) so k0 runs at the full 2.4 GHz clock.
        warm_last = None
        for _ in range(25):
            warm_last = warm_mm()

        def mm(out_ap, k, m, rhs_cols, start, stop):
            r = nc.tensor.matmul(
                out_ap,
                lhsT=b_sb[:, k, m * P:(m + 1) * P],
                rhs=xt_sb[:, k, rhs_cols[0]:rhs_cols[1]],
                start=start,
                stop=stop,
            )
            add_dep_helper(r.ins, warm_last.ins, sync=False)
            return r

        # --- phase A: m0..m2 k-outer (PE consumes ~1.3us per chunk, just
        # above the DMA delivery rate, so the PE stays busy and warm).
        for k in range(KC):
            st, sp = (k == 0), (k == KC - 1)
            for m in range(3):
                mm(ps[2 * m][:, :], k, m, (HALO, HALO + 512), st, sp)
                mm(ps[2 * m + 1][:, :], k, m, (HALO + 512, TLH), st, sp)
                mm(ps[6][:, 8 * m:8 * m + 8], k, m, (0, HALO), st, sp)
            if k < KC - 1:
                warm_mm()  # keep the HAM gate open across chunk-arrival gaps

        # --- phase A copies (scalar engine), ordered so the banks phase B
        # needs free up first: ps0 (B seg1), then the ps6 halo slivers
        # (B halo), then the rest; m0's three land first so its scans start
        # immediately.
        def cp(dst, src):
            nc.scalar.activation(dst, src, Copy)

        cp(s_sb[0][:, HALO:HALO + 512], ps[0][:, :])
        cp(s_sb[0][:, 0:HALO], ps[6][:, 0:8])
        cp(s_sb[0][:, HALO + 512:TLH], ps[1][:, :])
        cp(s_sb[1][:, 0:HALO], ps[6][:, 8:16])
        cp(s_sb[2][:, 0:HALO], ps[6][:, 16:24])
        cp(s_sb[1][:, HALO:HALO + 512], ps[2][:, :])
        cp(s_sb[1][:, HALO + 512:TLH], ps[3][:, :])
        cp(s_sb[2][:, HALO:HALO + 512], ps[4][:, :])
        cp(s_sb[2][:, HALO + 512:TLH], ps[5][:, :])

        # --- phase B: m3, seg-major (all chunks are resident by now).
        for k in range(KC):
            mm(ps[7][:, :], k, 3, (HALO, HALO + 512), k == 0, k == KC - 1)
        for k in range(KC):
            mm(ps[0][:, :], k, 3, (HALO + 512, TLH), k == 0, k == KC - 1)
        for k in range(KC):
            mm(ps[6][:, 24:32], k, 3, (0, HALO), k == 0, k == KC - 1)

        cp(s_sb[3][:, HALO:HALO + 512], ps[7][:, :])
        cp(s_sb[3][:, 0:HALO], ps[6][:, 24:32])
        cp(s_sb[3][:, HALO + 512:TLH], ps[0][:, :])

        # --- scans (DVE) + out DMAs (sync).  The first 8 columns warm the
        # carry from 0 and are discarded.  m0..m2: one 1032-col scan each
        # (per-scan fixed costs dominate splitting); m3 stays split lo/hi so
        # its first out DMA overlaps the final scan.
        scan = nc.vector.tensor_tensor_scan
        for m in range(3):
            scan(g_all[m][:, :], a_rep[m][:, :], s_sb[m][:, :], 0.0, MULT, ADD)
            nc.sync.dma_start(
                out=out_d[m * P:(m + 1) * P, :], in_=g_all[m][:, HALO:TLH]
            )
        scan(g_lo3[:, :], a_rep[3][:, 0:520], s_sb[3][:, 0:520], 0.0, MULT, ADD)
        nc.sync.dma_start(out=out_d[3 * P:4 * P, 0:512], in_=g_lo3[:, HALO:520])
        scan(g_hi3[:, :], a_rep[3][:, 0:512], s_sb[3][:, 520:TLH],
             g_lo3[:, 519:520], MULT, ADD)
        nc.sync.dma_start(out=out_d[3 * P:4 * P, 512:TL], in_=g_hi3[:, :])

    nc.compile()
    return nc


def _get_nc():
    if "nc" not in _CACHE:
        _CACHE["nc"] = _build_program()
    return _CACHE["nc"]


def _make_in_maps(x_seq, a_diag, b_mat):
    x_seq = np.ascontiguousarray(x_seq, dtype=np.float32)
    a_diag = np.asarray(a_diag, dtype=np.float32)
    b_mat = np.ascontiguousarray(b_mat, dtype=np.float32)

    # (H, HALO+T) in bf16: zero left-pad so every core reads [t0-8, t0+TL)
    xtp = np.concatenate(
        [np.zeros((H, HALO), np.float32), x_seq.T], axis=1
    ).astype(BF16)
    b16 = b_mat.astype(BF16)

    in_maps = []
    for c in range(N_CORES):
        ct, ch = divmod(c, NC_H)
        t0 = ct * TL
        h0 = ch * HL
        a_loc = a_diag[h0:h0 + HL].reshape(MT, P).T  # (128, MT)
        in_maps.append({
            "xt": np.ascontiguousarray(xtp[:, t0:t0 + TLH]),
            "bm": np.ascontiguousarray(b16[:, h0:h0 + HL]),
            "apd": np.ascontiguousarray(a_loc),
        })
    return in_maps


def _run(x_seq, a_diag, b_mat, trace=False):
    from concourse.bass_utils import run_bass_kernel_spmd

    nc = _get_nc()
    in_maps = _make_in_maps(x_seq, a_diag, b_mat)
    res = run_bass_kernel_spmd(nc, in_maps, list(range(N_CORES)), trace=trace)

    out = np.empty((T, H), np.float32)
    for c in range(N_CORES):
        ct, ch = divmod(c, NC_H)
        # per-core result is (HL, TL) bf16 in [h, t] layout
        blk = np.asarray(res.results[c]["out"], dtype=np.float32)
        out[ct * TL:(ct + 1) * TL, ch * HL:(ch + 1) * HL] = blk.T
    return out, res


def kernel(x_seq, a_diag, b_mat):
    out, _ = _run(x_seq, a_diag, b_mat, trace=False)
    return out


# revision 25
# speedup vs baseline: 1.1921x; 1.0657x over previous
"""Trainium2 Bass kernel for nn_DiagSSMBlock (T=4096, H=1024, fp32).

Math: s = b_mat.T @ x_seq.T  (H,T);  h[:, t] = a * h[:, t-1] + s[:, t]
      output = h.T  (T, H)

a_diag is glorot-scaled (|a| <= sqrt(2/1024) ~ 0.044): the power kernel decays
below fp32 epsilon within 8 taps, so an 8-step halo makes the T-sharded
recurrence exact to fp32 precision.  The 2e-2 rel-err budget also admits bf16
operands end to end (measured ~4e-3), which halves every DMA byte and doubles
matmul/weight-load throughput paths.

Sharding (8 cores): 4-way along T x 2-way along H_out.
Per core:
  GEMM   (1032 t) x (512 h_out) x (1024 contract) in bf16 via PE matmuls,
         k-outer over h-tiles {m0,m1,m2} while input chunks stream, then m3
         (PSUM: 8 banks = 3x2 segs + halo slivers + warm/B-phase reuse).
  copies PSUM fp32 -> SBUF bf16 on the scalar engine.
  scan   DVE tensor_tensor_scan per h-tile, lo half chained into hi half via
         a tensor `initial` (halo columns warm the carry from zero).
  out    [h, t] layout DMA'd straight from the scan output -- no transposes;
         the host transposes each core's (512, 1024) block while unsharding.
"""

import sys

import numpy as np

if "/opt/trn_rl_repo" not in sys.path:
    sys.path.insert(0, "/opt/trn_rl_repo")

import ml_dtypes

BF16 = ml_dtypes.bfloat16

T, H = 4096, 1024
NC_T, NC_H = 4, 2  # core grid: 4 T-shards x 2 H-shards
TL = T // NC_T  # 1024 output rows per core
HL = H // NC_H  # 512 output cols per core
HALO = 8  # recurrence warm-up steps
TLH = TL + HALO  # 1032
P = 128
KC = H // P  # 8 contraction chunks
MT = HL // P  # 4 h_out tiles per core
N_CORES = NC_T * NC_H

_CACHE = {}


def _build_program():
    from contextlib import ExitStack

    import concourse.bass as bass
    import concourse.tile as tile
    from concourse import bacc, mybir

    f32 = mybir.dt.float32
    bf16 = mybir.dt.bfloat16
    Copy = mybir.ActivationFunctionType.Copy
    ADD = mybir.AluOpType.add
    MULT = mybir.AluOpType.mult

    nc = bacc.Bacc("TRN2", target_bir_lowering=False, debug=False, num_devices=N_CORES)

    xt_d = nc.dram_tensor("xt", [H, TLH], bf16, kind="ExternalInput").ap()
    b_d = nc.dram_tensor("bm", [H, HL], bf16, kind="ExternalInput").ap()
    a_d = nc.dram_tensor("apd", [P, MT], f32, kind="ExternalInput").ap()
    out_d = nc.dram_tensor("out", [HL, TL], bf16, kind="ExternalOutput").ap()

    from concourse.tile import add_dep_helper

    with tile.TileContext(nc) as tc, ExitStack() as ctx:
        const = ctx.enter_context(tc.tile_pool(name="const", bufs=1))
        s_pool = ctx.enter_context(tc.tile_pool(name="s", bufs=1))
        g_pool = ctx.enter_context(tc.tile_pool(name="g", bufs=1))
        psum = ctx.enter_context(tc.tile_pool(name="psfix", bufs=1, space="PSUM"))

        xt_sb = const.tile([P, KC, TLH], bf16)
        b_sb = const.tile([P, KC, HL], bf16)
        a_raw = const.tile([P, MT], f32)
        w_sb = const.tile([P, P], bf16)
        a_rep = [const.tile([P, TLH], bf16, name=f"arep{m}") for m in range(MT)]
        s_sb = [s_pool.tile([P, TLH], bf16, name=f"s{m}") for m in range(MT)]
        g_all = [g_pool.tile([P, TLH], bf16, name=f"g{m}") for m in range(3)]
        g_lo3 = g_pool.tile([P, 520], bf16, name="glo3")
        g_hi3 = g_pool.tile([P, 512], bf16, name="ghi3")

        # warm-matmul operand: on-chip memset, no DMA dependency, so the PE
        # warm-up chain starts the moment the engines come up.
        nc.vector.memset(w_sb[:, :], 0.0)

        # --- input DMAs.  x chunks on sync, b chunks on scalar: two HWDGE
        # FIFO streams drain in issue order, so chunk k lands ~k*1.45us in and
        # the k-outer GEMM consumes right behind the stream.  (Measured: a
        # third SWDGE stream, multi-chunk groups, and partition-major layouts
        # all deliver SLOWER than this simple scheme.)  apd rides SWDGE on
        # the otherwise-idle GpSimd sequencer.
        nc.gpsimd.dma_start(out=a_raw[:, :], in_=a_d[:, :])
        for k in range(KC):
            nc.sync.dma_start(out=xt_sb[:, k, :], in_=xt_d[k * P:(k + 1) * P, :])
            nc.scalar.dma_start(out=b_sb[:, k, :], in_=b_d[k * P:(k + 1) * P, :])

        # a broadcast in bf16, materialized (packed last dim) while DVE is
        # otherwise idle.
        for m in range(MT):
            nc.vector.tensor_copy(
                a_rep[m][:, :], a_raw[:, m:m + 1].broadcast_to([P, TLH])
            )

        ps = [psum.tile([P, 512], f32, tag=f"ps{i}", name=f"ps{i}") for i in range(8)]
        # bank plan: phase A (m0..m2): segs -> ps[2m], ps[2m+1]; halos -> ps6
        # cols [8m:8m+8].  warm matmuls + phase B (m3): seg0 -> ps7,
        # seg1 -> ps0 (freed by first copy), halo -> ps6 cols [24:32].

        # --- PE warmup while the first chunks stream: flips the HAM
        # clock-gate to 8/8 before the real GEMM.
        def warm_mm():
            return nc.tensor.matmul(
                ps[7][0:P, 0:P], lhsT=w_sb[:, :], rhs=w_sb[:, :],
                start=True, stop=True,
            )

        # ~25 warm matmuls bridge from engine-up (~7.5us) to chunk0 arrival
        # (~11.5us) so k0 runs at the full 2.4 GHz clock.
        warm_last = None
        for _ in range(25):
            warm_last = warm_mm()

        def mm(out_ap, k, m, rhs_cols, start, stop):
            r = nc.tensor.matmul(
                out_ap,
                lhsT=b_sb[:, k, m * P:(m + 1) * P],
                rhs=xt_sb[:, k, rhs_cols[0]:rhs_cols[1]],
                start=start,
                stop=stop,
            )
            add_dep_helper(r.ins, warm_last.ins, sync=False)
            return r

        # --- phase A: m0..m2 k-outer (PE consumes ~1.3us per chunk, just
        # above the DMA delivery rate, so the PE stays busy and warm).
        for k in range(KC):
            st, sp = (k == 0), (k == KC - 1)
            for m in range(3):
                mm(ps[2 * m][:, :], k, m, (HALO, HALO + 512), st, sp)
                mm(ps[2 * m + 1][:, :], k, m, (HALO + 512, TLH), st, sp)
                mm(ps[6][:, 8 * m:8 * m + 8], k, m, (0, HALO), st, sp)
            if k < KC - 1:
                warm_mm()  # keep the HAM gate open across chunk-arrival gaps

        # --- phase A copies (scalar engine), ordered so the banks phase B
        # needs free up first: ps0 (B seg1), then the ps6 halo slivers
        # (B halo), then the rest; m0's three land first so its scans start
        # immediately.
        def cp(dst, src):
            nc.scalar.activation(dst, src, Copy)

        cp(s_sb[0][:, HALO:HALO + 512], ps[0][:, :])
        cp(s_sb[0][:, 0:HALO], ps[6][:, 0:8])
        cp(s_sb[0][:, HALO + 512:TLH], ps[1][:, :])
        cp(s_sb[1][:, 0:HALO], ps[6][:, 8:16])
        cp(s_sb[2][:, 0:HALO], ps[6][:, 16:24])
        cp(s_sb[1][:, HALO:HALO + 512], ps[2][:, :])
        cp(s_sb[1][:, HALO + 512:TLH], ps[3][:, :])
        cp(s_sb[2][:, HALO:HALO + 512], ps[4][:, :])
        cp(s_sb[2][:, HALO + 512:TLH], ps[5][:, :])

        # --- phase B: m3, seg-major (all chunks are resident by now).
        for k in range(KC):
            mm(ps[7][:, :], k, 3, (HALO, HALO + 512), k == 0, k == KC - 1)
        for k in range(KC):
            mm(ps[0][:, :], k, 3, (HALO + 512, TLH), k == 0, k == KC - 1)
        for k in range(KC):
            mm(ps[6][:, 24:32], k, 3, (0, HALO), k == 0, k == KC - 1)

        cp(s_sb[3][:, HALO:HALO + 512], ps[7][:, :])
        cp(s_sb[3][:, 0:HALO], ps[6][:, 24:32])
        cp(s_sb[3][:, HALO + 512:TLH], ps[0][:, :])

        # --- scans (DVE) + out DMAs (sync).  The first 8 columns warm the
        # carry from 0 and are discarded.  m0..m2: one 1032-col scan each
        # (per-scan fixed costs dominate splitting); m3 stays split lo/hi so
        # its first out DMA overlaps the final scan.
        scan = nc.vector.tensor_tensor_scan
        for m in range(3):
            scan(g_all[m][:, :], a_rep[m][:, :], s_sb[m][:, :], 0.0, MULT, ADD)
            nc.sync.dma_start(
                out=out_d[m * P:(m + 1) * P, :], in_=g_all[m][:, HALO:TLH]
            )
        scan(g_lo3[:, :], a_rep[3][:, 0:520], s_sb[3][:, 0:520], 0.0, MULT, ADD)
        nc.sync.dma_start(out=out_d[3 * P:4 * P, 0:512], in_=g_lo3[:, HALO:520])
        scan(g_hi3[:, :], a_rep[3][:, 0:512], s_sb[3][:, 520:TLH],
             g_lo3[:, 519:520], MULT, ADD)
        nc.sync.dma_start(out=out_d[3 * P:4 * P, 512:TL], in_=g_hi3[:, :])

    nc.compile()
    return nc


def _get_nc():
    if "nc" not in _CACHE:
        _CACHE["nc"] = _build_program()
    return _CACHE["nc"]


def _make_in_maps(x_seq, a_diag, b_mat):
    x_seq = np.ascontiguousarray(x_seq, dtype=np.float32)
    a_diag = np.asarray(a_diag, dtype=np.float32)
    b_mat = np.ascontiguousarray(b_mat, dtype=np.float32)

    # (H, HALO+T) in bf16: zero left-pad so every core reads [t0-8, t0+TL)
    xtp = np.concatenate(
        [np.zeros((H, HALO), np.float32), x_seq.T], axis=1
    ).astype(BF16)
    b16 = b_mat.astype(BF16)

    in_maps = []
    for c in range(N_CORES):
        ct, ch = divmod(c, NC_H)
        t0 = ct * TL
        h0 = ch * HL
        a_loc = a_diag[h0:h0 + HL].reshape(MT, P).T  # (128, MT)
        in_maps.append({
            "xt": np.ascontiguousarray(xtp[:, t0:t0 + TLH]),
            "bm": np.ascontiguousarray(b16[:, h0:h0 + HL]),
            "apd": np.ascontiguousarray(a_loc),
        })
    return in_maps


def _run(x_seq, a_diag, b_mat, trace=False):
    from concourse.bass_utils import run_bass_kernel_spmd

    nc = _get_nc()
    in_maps = _make_in_maps(x_seq, a_diag, b_mat)
    res = run_bass_kernel_spmd(nc, in_maps, list(range(N_CORES)), trace=trace)

    out = np.empty((T, H), np.float32)
    for c in range(N_CORES):
        ct, ch = divmod(c, NC_H)
        # per-core result is (HL, TL) bf16 in [h, t] layout
        blk = np.asarray(res.results[c]["out"], dtype=np.float32)
        out[ct * TL:(ct + 1) * TL, ch * HL:(ch + 1) * HL] = blk.T
    return out, res


def kernel(x_seq, a_diag, b_mat):
    out, _ = _run(x_seq, a_diag, b_mat, trace=False)
    return out


# revision 30
# speedup vs baseline: 1.2391x; 1.0394x over previous
"""Trainium2 Bass kernel for nn_DiagSSMBlock (T=4096, H=1024, fp32).

Math: s = b_mat.T @ x_seq.T  (H,T);  h[:, t] = a * h[:, t-1] + s[:, t]
      output = h.T  (T, H)

a_diag is glorot-scaled (|a| <= sqrt(2/1024) ~ 0.044): the power kernel decays
below fp32 epsilon within 8 taps, so an 8-step halo makes the T-sharded
recurrence exact to fp32 precision.  The 2e-2 rel-err budget also admits bf16
operands end to end (measured ~4e-3), which halves every DMA byte and doubles
matmul/weight-load throughput paths.

Sharding (8 cores): 4-way along T x 2-way along H_out.
Per core:
  GEMM   (1032 t) x (512 h_out) x (1024 contract) in bf16 via PE matmuls,
         k-outer over h-tiles {m0,m1,m2} while input chunks stream, then m3
         (PSUM: 8 banks = 3x2 segs + halo slivers + warm/B-phase reuse).
  copies PSUM fp32 -> SBUF bf16 on the scalar engine.
  scan   DVE tensor_tensor_scan per h-tile, lo half chained into hi half via
         a tensor `initial` (halo columns warm the carry from zero).
  out    [h, t] layout DMA'd straight from the scan output -- no transposes;
         the host transposes each core's (512, 1024) block while unsharding.
"""

import sys

import numpy as np

if "/opt/trn_rl_repo" not in sys.path:
    sys.path.insert(0, "/opt/trn_rl_repo")

import ml_dtypes

BF16 = ml_dtypes.bfloat16

T, H = 4096, 1024
NC_T, NC_H = 4, 2  # core grid: 4 T-shards x 2 H-shards
TL = T // NC_T  # 1024 output rows per core
HL = H // NC_H  # 512 output cols per core
HALO = 8  # recurrence warm-up steps
TLH = TL + HALO  # 1032
P = 128
KC = H // P  # 8 contraction chunks
MT = HL // P  # 4 h_out tiles per core
N_CORES = NC_T * NC_H

_CACHE = {}


def _build_program():
    from contextlib import ExitStack

    import concourse.bass as bass
    import concourse.tile as tile
    from concourse import bacc, mybir

    f32 = mybir.dt.float32
    bf16 = mybir.dt.bfloat16
    Copy = mybir.ActivationFunctionType.Copy
    ADD = mybir.AluOpType.add
    MULT = mybir.AluOpType.mult

    nc = bacc.Bacc("TRN2", target_bir_lowering=False, debug=False, num_devices=N_CORES)

    # x and b fused row-wise: row hin = [x^T[hin, t0-8:t0+TL] | b[hin, h0:h0+HL]]
    # so chunk k of BOTH loads in ONE DMA (128 rows x 3088B contiguous).
    XBW = TLH + HL  # 2056
    xb_d = nc.dram_tensor("xb", [H, XBW], bf16, kind="ExternalInput").ap()
    a_d = nc.dram_tensor("apd", [P, MT], f32, kind="ExternalInput").ap()
    out_d = nc.dram_tensor("out", [HL, TL], bf16, kind="ExternalOutput").ap()

    from concourse.tile import add_dep_helper

    with tile.TileContext(nc) as tc, ExitStack() as ctx:
        const = ctx.enter_context(tc.tile_pool(name="const", bufs=1))
        s_pool = ctx.enter_context(tc.tile_pool(name="s", bufs=1))
        g_pool = ctx.enter_context(tc.tile_pool(name="g", bufs=1))
        psum = ctx.enter_context(tc.tile_pool(name="psfix", bufs=1, space="PSUM"))

        xb_sb = const.tile([P, KC, XBW], bf16)
        a_raw = const.tile([P, MT], f32)
        w_sb = const.tile([P, P], bf16)
        a_rep = [const.tile([P, TLH], bf16, name=f"arep{m}") for m in range(MT)]
        s_sb = [s_pool.tile([P, TLH], bf16, name=f"s{m}") for m in range(MT)]
        g_all = [g_pool.tile([P, TLH], bf16, name=f"g{m}") for m in range(3)]
        g_lo3 = g_pool.tile([P, 520], bf16, name="glo3")
        g_hi3 = g_pool.tile([P, 512], bf16, name="ghi3")

        # warm-matmul operand: on-chip memset, no DMA dependency, so the PE
        # warm-up chain starts the moment the engines come up.
        nc.vector.memset(w_sb[:, :], 0.0)

        # --- input DMAs: one fused x+b DMA per contraction chunk (half the
        # completions/receipts of separate streams, 3KB contiguous
        # descriptors), alternating across the two HWDGE rings so two chunks
        # are always in flight.  apd rides SWDGE on the idle GpSimd
        # sequencer.  (Measured dead ends: a third SWDGE stream, multi-chunk
        # groups, and partition-major layouts all deliver slower.)
        nc.gpsimd.dma_start(out=a_raw[:, :], in_=a_d[:, :])
        for k in range(KC):
            eng = nc.sync if k % 2 == 0 else nc.scalar
            eng.dma_start(out=xb_sb[:, k, :], in_=xb_d[k * P:(k + 1) * P, :])

        # a broadcast in bf16, materialized (packed last dim) while DVE is
        # otherwise idle.
        for m in range(MT):
            nc.vector.tensor_copy(
                a_rep[m][:, :], a_raw[:, m:m + 1].broadcast_to([P, TLH])
            )

        ps = [psum.tile([P, 512], f32, tag=f"ps{i}", name=f"ps{i}") for i in range(8)]
        # bank plan: phase A (m0..m2): segs -> ps[2m], ps[2m+1]; halos -> ps6
        # cols [8m:8m+8].  warm matmuls + phase B (m3): seg0 -> ps7,
        # seg1 -> ps0 (freed by first copy), halo -> ps6 cols [24:32].

        # --- PE warmup while the first chunks stream: flips the HAM
        # clock-gate to 8/8 before the real GEMM.
        def warm_mm():
            return nc.tensor.matmul(
                ps[7][0:P, 0:P], lhsT=w_sb[:, :], rhs=w_sb[:, :],
                start=True, stop=True,
            )

        # warm matmuls bridge from engine-up (~7.5us) to chunk0 arrival
        # (~11us), long enough (>3.4us) to flip the HAM gate so k0 runs at
        # the full 2.4 GHz clock.
        warm_last = None
        for _ in range(32):
            warm_last = warm_mm()

        def mm(out_ap, k, m, rhs_cols, start, stop):
            r = nc.tensor.matmul(
                out_ap,
                lhsT=xb_sb[:, k, TLH + m * P:TLH + (m + 1) * P],
                rhs=xb_sb[:, k, rhs_cols[0]:rhs_cols[1]],
                start=start,
                stop=stop,
            )
            add_dep_helper(r.ins, warm_last.ins, sync=False)
            return r

        # --- phase A: m0..m2 k-outer (PE consumes ~1.3us per chunk, just
        # above the DMA delivery rate, so the PE stays busy and warm).
        for k in range(KC):
            st, sp = (k == 0), (k == KC - 1)
            for m in range(3):
                mm(ps[2 * m][:, :], k, m, (HALO, HALO + 512), st, sp)
                mm(ps[2 * m + 1][:, :], k, m, (HALO + 512, TLH), st, sp)
                mm(ps[6][:, 8 * m:8 * m + 8], k, m, (0, HALO), st, sp)
            if k < KC - 1:
                warm_mm()  # keep the HAM gate open across chunk-arrival gaps

        # --- phase A copies (scalar engine), ordered so the banks phase B
        # needs free up first: ps0 (B seg1), then the ps6 halo slivers
        # (B halo), then the rest; m0's three land first so its scans start
        # immediately.
        def cp(dst, src):
            nc.scalar.activation(dst, src, Copy)

        cp(s_sb[0][:, HALO:HALO + 512], ps[0][:, :])
        cp(s_sb[0][:, 0:HALO], ps[6][:, 0:8])
        cp(s_sb[0][:, HALO + 512:TLH], ps[1][:, :])
        cp(s_sb[1][:, 0:HALO], ps[6][:, 8:16])
        cp(s_sb[2][:, 0:HALO], ps[6][:, 16:24])
        cp(s_sb[1][:, HALO:HALO + 512], ps[2][:, :])
        cp(s_sb[1][:, HALO + 512:TLH], ps[3][:, :])
        cp(s_sb[2][:, HALO:HALO + 512], ps[4][:, :])
        cp(s_sb[2][:, HALO + 512:TLH], ps[5][:, :])

        # --- phase B: m3, seg-major (all chunks are resident by now).
        for k in range(KC):
            mm(ps[7][:, :], k, 3, (HALO, HALO + 512), k == 0, k == KC - 1)
        for k in range(KC):
            mm(ps[0][:, :], k, 3, (HALO + 512, TLH), k == 0, k == KC - 1)
        for k in range(KC):
            mm(ps[6][:, 24:32], k, 3, (0, HALO), k == 0, k == KC - 1)

        cp(s_sb[3][:, HALO:HALO + 512], ps[7][:, :])
        cp(s_sb[3][:, 0:HALO], ps[6][:, 24:32])
        cp(s_sb[3][:, HALO + 512:TLH], ps[0][:, :])

        # --- scans (DVE) + out DMAs (sync).  The first 8 columns warm the
        # carry from 0 and are discarded.  m0..m2: one 1032-col scan each
        # (per-scan fixed costs dominate splitting); m3 stays split lo/hi so
        # its first out DMA overlaps the final scan.
        scan = nc.vector.tensor_tensor_scan
        for m in range(3):
            scan(g_all[m][:, :], a_rep[m][:, :], s_sb[m][:, :], 0.0, MULT, ADD)
            nc.sync.dma_start(
                out=out_d[m * P:(m + 1) * P, :], in_=g_all[m][:, HALO:TLH]
            )
        scan(g_lo3[:, :], a_rep[3][:, 0:520], s_sb[3][:, 0:520], 0.0, MULT, ADD)
        nc.sync.dma_start(out=out_d[3 * P:4 * P, 0:512], in_=g_lo3[:, HALO:520])
        scan(g_hi3[:, :], a_rep[3][:, 0:512], s_sb[3][:, 520:TLH],
             g_lo3[:, 519:520], MULT, ADD)
        nc.sync.dma_start(out=out_d[3 * P:4 * P, 512:TL], in_=g_hi3[:, :])

    nc.compile()
    return nc


def _get_nc():
    if "nc" not in _CACHE:
        _CACHE["nc"] = _build_program()
    return _CACHE["nc"]


def _make_in_maps(x_seq, a_diag, b_mat):
    x_seq = np.ascontiguousarray(x_seq, dtype=np.float32)
    a_diag = np.asarray(a_diag, dtype=np.float32)
    b_mat = np.ascontiguousarray(b_mat, dtype=np.float32)

    # (H, HALO+T) in bf16: zero left-pad so every core reads [t0-8, t0+TL)
    xtp = np.concatenate(
        [np.zeros((H, HALO), np.float32), x_seq.T], axis=1
    ).astype(BF16)
    b16 = b_mat.astype(BF16)

    in_maps = []
    for c in range(N_CORES):
        ct, ch = divmod(c, NC_H)
        t0 = ct * TL
        h0 = ch * HL
        a_loc = a_diag[h0:h0 + HL].reshape(MT, P).T  # (128, MT)
        # fused row: [x^T slice | b slice] so chunk k of both is one DMA
        xb = np.concatenate(
            [xtp[:, t0:t0 + TLH], b16[:, h0:h0 + HL]], axis=1
        )
        in_maps.append({
            "xb": np.ascontiguousarray(xb),
            "apd": np.ascontiguousarray(a_loc),
        })
    return in_maps


def _run(x_seq, a_diag, b_mat, trace=False):
    from concourse.bass_utils import run_bass_kernel_spmd

    nc = _get_nc()
    in_maps = _make_in_maps(x_seq, a_diag, b_mat)
    res = run_bass_kernel_spmd(nc, in_maps, list(range(N_CORES)), trace=trace)

    out = np.empty((T, H), np.float32)
    for c in range(N_CORES):
        ct, ch = divmod(c, NC_H)
        # per-core result is (HL, TL) bf16 in [h, t] layout
        blk = np.asarray(res.results[c]["out"], dtype=np.float32)
        out[ct * TL:(ct + 1) * TL, ch * HL:(ch + 1) * HL] = blk.T
    return out, res


def kernel(x_seq, a_diag, b_mat):
    out, _ = _run(x_seq, a_diag, b_mat, trace=False)
    return out


# revision 33
# speedup vs baseline: 1.2751x; 1.0290x over previous
"""Trainium2 Bass kernel for nn_DiagSSMBlock (T=4096, H=1024, fp32).

Math: s = b_mat.T @ x_seq.T  (H,T);  h[:, t] = a * h[:, t-1] + s[:, t]
      output = h.T  (T, H)

a_diag is glorot-scaled (|a| <= sqrt(2/1024) ~ 0.044): the power kernel decays
below fp32 epsilon within 8 taps, so an 8-step halo makes the T-sharded
recurrence exact to fp32 precision.  The 2e-2 rel-err budget also admits bf16
operands end to end (measured ~4e-3), which halves every DMA byte and doubles
matmul/weight-load throughput paths.

Sharding (8 cores): 4-way along T x 2-way along H_out.
Per core:
  GEMM   (1032 t) x (512 h_out) x (1024 contract) in bf16 via PE matmuls,
         k-outer over h-tiles {m0,m1,m2} while input chunks stream, then m3
         (PSUM: 8 banks = 3x2 segs + halo slivers + warm/B-phase reuse).
  copies PSUM fp32 -> SBUF bf16 on the scalar engine.
  scan   DVE tensor_tensor_scan per h-tile, lo half chained into hi half via
         a tensor `initial` (halo columns warm the carry from zero).
  out    [h, t] layout DMA'd straight from the scan output -- no transposes;
         the host transposes each core's (512, 1024) block while unsharding.
"""

import sys

import numpy as np

if "/opt/trn_rl_repo" not in sys.path:
    sys.path.insert(0, "/opt/trn_rl_repo")

import ml_dtypes

BF16 = ml_dtypes.bfloat16

T, H = 4096, 1024
NC_T, NC_H = 4, 2  # core grid: 4 T-shards x 2 H-shards
TL = T // NC_T  # 1024 output rows per core
HL = H // NC_H  # 512 output cols per core
HALO = 8  # recurrence warm-up steps
TLH = TL + HALO  # 1032
P = 128
KC = H // P  # 8 contraction chunks
MT = HL // P  # 4 h_out tiles per core
N_CORES = NC_T * NC_H

_CACHE = {}


def _build_program():
    from contextlib import ExitStack

    import concourse.bass as bass
    import concourse.tile as tile
    from concourse import bacc, mybir

    f32 = mybir.dt.float32
    bf16 = mybir.dt.bfloat16
    Copy = mybir.ActivationFunctionType.Copy
    ADD = mybir.AluOpType.add
    MULT = mybir.AluOpType.mult

    nc = bacc.Bacc("TRN2", target_bir_lowering=False, debug=False, num_devices=N_CORES)

    # x and b fused row-wise: row hin = [x^T[hin, t0-8:t0+TL] | b[hin, h0:h0+HL]]
    # so chunk k of BOTH loads in ONE DMA (128 rows x 3088B contiguous).
    XBW = TLH + HL  # 2056
    xb_d = nc.dram_tensor("xb", [H, XBW], bf16, kind="ExternalInput").ap()
    a_d = nc.dram_tensor("apd", [P, MT], f32, kind="ExternalInput").ap()
    out_d = nc.dram_tensor("out", [HL, TL], bf16, kind="ExternalOutput").ap()

    from concourse.tile import add_dep_helper

    with tile.TileContext(nc) as tc, ExitStack() as ctx:
        const = ctx.enter_context(tc.tile_pool(name="const", bufs=1))
        s_pool = ctx.enter_context(tc.tile_pool(name="s", bufs=1))
        g_pool = ctx.enter_context(tc.tile_pool(name="g", bufs=1))
        psum = ctx.enter_context(tc.tile_pool(name="psfix", bufs=1, space="PSUM"))

        xb_sb = const.tile([P, KC, XBW], bf16)
        a_raw = const.tile([P, MT], f32)
        w_sb = const.tile([P, P], bf16)
        a_rep = [const.tile([P, TLH], bf16, name=f"arep{m}") for m in range(MT)]
        s_sb = [s_pool.tile([P, TLH], bf16, name=f"s{m}") for m in range(MT)]
        g_lo0 = g_pool.tile([P, 520], bf16, name="glo0")
        g_hi0 = g_pool.tile([P, 512], bf16, name="ghi0")
        g_all = [g_pool.tile([P, TLH], bf16, name=f"g{m}") for m in (1, 2)]
        g_lo3 = g_pool.tile([P, 520], bf16, name="glo3")
        g_hi3 = g_pool.tile([P, 512], bf16, name="ghi3")

        # warm-matmul operand: on-chip memset, no DMA dependency, so the PE
        # warm-up chain starts the moment the engines come up.
        nc.vector.memset(w_sb[:, :], 0.0)

        # --- input DMAs: one fused x+b DMA per contraction chunk (half the
        # completions/receipts of separate streams, 3KB contiguous
        # descriptors), alternating across the two HWDGE rings so two chunks
        # are always in flight.  apd rides SWDGE on the idle GpSimd
        # sequencer.  (Measured dead ends: a third SWDGE stream, multi-chunk
        # groups, and partition-major layouts all deliver slower.)
        nc.gpsimd.dma_start(out=a_raw[:, :], in_=a_d[:, :])
        for k in range(KC):
            eng = nc.sync if k % 2 == 0 else nc.scalar
            eng.dma_start(out=xb_sb[:, k, :], in_=xb_d[k * P:(k + 1) * P, :])

        # a broadcast in bf16, materialized (packed last dim) while DVE is
        # otherwise idle.
        for m in range(MT):
            nc.vector.tensor_copy(
                a_rep[m][:, :], a_raw[:, m:m + 1].broadcast_to([P, TLH])
            )

        ps = [psum.tile([P, 512], f32, tag=f"ps{i}", name=f"ps{i}") for i in range(8)]
        # bank plan: phase A (m0..m2): segs -> ps[2m], ps[2m+1]; halos -> ps6
        # cols [8m:8m+8].  warm matmuls + phase B (m3): seg0 -> ps7,
        # seg1 -> ps0 (freed by first copy), halo -> ps6 cols [24:32].

        # --- PE warmup while the first chunks stream: flips the HAM
        # clock-gate to 8/8 before the real GEMM.
        def warm_mm():
            return nc.tensor.matmul(
                ps[7][0:P, 0:P], lhsT=w_sb[:, :], rhs=w_sb[:, :],
                start=True, stop=True,
            )

        # warm matmuls bridge from engine-up (~7.5us) to chunk0 arrival
        # (~11us), long enough (>3.4us) to flip the HAM gate so k0 runs at
        # the full 2.4 GHz clock.
        warm_last = None
        for _ in range(32):
            warm_last = warm_mm()

        def mm(out_ap, k, m, rhs_cols, start, stop):
            r = nc.tensor.matmul(
                out_ap,
                lhsT=xb_sb[:, k, TLH + m * P:TLH + (m + 1) * P],
                rhs=xb_sb[:, k, rhs_cols[0]:rhs_cols[1]],
                start=start,
                stop=stop,
            )
            add_dep_helper(r.ins, warm_last.ins, sync=False)
            return r

        # --- phase A: m0..m2 k-outer (PE consumes ~1.3us per chunk, just
        # above the DMA delivery rate, so the PE stays busy and warm).
        for k in range(KC):
            st, sp = (k == 0), (k == KC - 1)
            for m in range(3):
                mm(ps[2 * m][:, :], k, m, (HALO, HALO + 512), st, sp)
                mm(ps[2 * m + 1][:, :], k, m, (HALO + 512, TLH), st, sp)
                mm(ps[6][:, 8 * m:8 * m + 8], k, m, (0, HALO), st, sp)
            if k < KC - 1:
                warm_mm()  # keep the HAM gate open across chunk-arrival gaps

        # --- phase A copies.  m0's lo-half inputs (seg0 + halo) are copied
        # by the DVE itself: same-engine program order feeds its first scan
        # with no ACT write-ack + semaphore hop.  ACT concurrently copies
        # m0's hi seg and everything else, ordered so the banks phase B
        # needs (ps0, ps6) free up first.
        def cp(dst, src):
            nc.scalar.activation(dst, src, Copy)

        nc.vector.tensor_copy(s_sb[0][:, HALO:HALO + 512], ps[0][:, :])
        nc.vector.tensor_copy(s_sb[0][:, 0:HALO], ps[6][:, 0:8])
        cp(s_sb[0][:, HALO + 512:TLH], ps[1][:, :])
        cp(s_sb[1][:, 0:HALO], ps[6][:, 8:16])
        cp(s_sb[2][:, 0:HALO], ps[6][:, 16:24])
        cp(s_sb[1][:, HALO:HALO + 512], ps[2][:, :])
        cp(s_sb[1][:, HALO + 512:TLH], ps[3][:, :])
        cp(s_sb[2][:, HALO:HALO + 512], ps[4][:, :])
        cp(s_sb[2][:, HALO + 512:TLH], ps[5][:, :])

        # --- phase B: m3, seg-major (all chunks are resident by now).
        for k in range(KC):
            mm(ps[7][:, :], k, 3, (HALO, HALO + 512), k == 0, k == KC - 1)
        for k in range(KC):
            mm(ps[0][:, :], k, 3, (HALO + 512, TLH), k == 0, k == KC - 1)
        for k in range(KC):
            mm(ps[6][:, 24:32], k, 3, (0, HALO), k == 0, k == KC - 1)

        cp(s_sb[3][:, HALO:HALO + 512], ps[7][:, :])
        cp(s_sb[3][:, 0:HALO], ps[6][:, 24:32])
        cp(s_sb[3][:, HALO + 512:TLH], ps[0][:, :])

        # --- scans (DVE) + out DMAs (sync).  The first 8 columns warm the
        # carry from 0 and are discarded.  m0 is split lo/hi: its lo scan
        # follows the DVE copies in program order with no cross-engine gate.
        # m1/m2: one 1032-col scan each (per-scan fixed costs dominate
        # splitting); m3 split lo/hi so its first out overlaps the last scan.
        scan = nc.vector.tensor_tensor_scan
        scan(g_lo0[:, :], a_rep[0][:, 0:520], s_sb[0][:, 0:520], 0.0, MULT, ADD)
        nc.sync.dma_start(out=out_d[0:P, 0:512], in_=g_lo0[:, HALO:520])
        scan(g_hi0[:, :], a_rep[0][:, 0:512], s_sb[0][:, 520:TLH],
             g_lo0[:, 519:520], MULT, ADD)
        nc.sync.dma_start(out=out_d[0:P, 512:TL], in_=g_hi0[:, :])
        for i, m in enumerate((1, 2)):
            scan(g_all[i][:, :], a_rep[m][:, :], s_sb[m][:, :], 0.0, MULT, ADD)
            nc.sync.dma_start(
                out=out_d[m * P:(m + 1) * P, :], in_=g_all[i][:, HALO:TLH]
            )
        scan(g_lo3[:, :], a_rep[3][:, 0:520], s_sb[3][:, 0:520], 0.0, MULT, ADD)
        nc.sync.dma_start(out=out_d[3 * P:4 * P, 0:512], in_=g_lo3[:, HALO:520])
        scan(g_hi3[:, :], a_rep[3][:, 0:512], s_sb[3][:, 520:TLH],
             g_lo3[:, 519:520], MULT, ADD)
        nc.sync.dma_start(out=out_d[3 * P:4 * P, 512:TL], in_=g_hi3[:, :])

    nc.compile()
    return nc


def _get_nc():
    if "nc" not in _CACHE:
        _CACHE["nc"] = _build_program()
    return _CACHE["nc"]


def _make_in_maps(x_seq, a_diag, b_mat):
    x_seq = np.ascontiguousarray(x_seq, dtype=np.float32)
    a_diag = np.asarray(a_diag, dtype=np.float32)
    b_mat = np.ascontiguousarray(b_mat, dtype=np.float32)

    # (H, HALO+T) in bf16: zero left-pad so every core reads [t0-8, t0+TL)
    xtp = np.concatenate(
        [np.zeros((H, HALO), np.float32), x_seq.T], axis=1
    ).astype(BF16)
    b16 = b_mat.astype(BF16)

    in_maps = []
    for c in range(N_CORES):
        ct, ch = divmod(c, NC_H)
        t0 = ct * TL
        h0 = ch * HL
        a_loc = a_diag[h0:h0 + HL].reshape(MT, P).T  # (128, MT)
        # fused row: [x^T slice | b slice] so chunk k of both is one DMA
        xb = np.concatenate(
            [xtp[:, t0:t0 + TLH], b16[:, h0:h0 + HL]], axis=1
        )
        in_maps.append({
            "xb": np.ascontiguousarray(xb),
            "apd": np.ascontiguousarray(a_loc),
        })
    return in_maps


def _run(x_seq, a_diag, b_mat, trace=False):
    from concourse.bass_utils import run_bass_kernel_spmd

    nc = _get_nc()
    in_maps = _make_in_maps(x_seq, a_diag, b_mat)
    res = run_bass_kernel_spmd(nc, in_maps, list(range(N_CORES)), trace=trace)

    out = np.empty((T, H), np.float32)
    for c in range(N_CORES):
        ct, ch = divmod(c, NC_H)
        # per-core result is (HL, TL) bf16 in [h, t] layout
        blk = np.asarray(res.results[c]["out"], dtype=np.float32)
        out[ct * TL:(ct + 1) * TL, ch * HL:(ch + 1) * HL] = blk.T
    return out, res


def kernel(x_seq, a_diag, b_mat):
    out, _ = _run(x_seq, a_diag, b_mat, trace=False)
    return out
